# revision 36
# baseline (speedup 1.0000x reference)
"""Sliding-window attention (B=2, S=2048, D=2048, H=16, HD=128, W=256) on 8
Trainium2 NeuronCores.

Sharding: data-parallel on batch (2) x sequence-parallel (4 chunks of 512
queries). Each core recomputes the K/V projections for its 256-position halo,
so there are no collectives; the host gathers the 8 output slices.

Per-core pipeline (all matmuls bf16 with f32 PSUM accumulation):
  1. V = x @ wv.T      (x-stationary, output in [seq, feat] layout)
  2. K,Q = x @ w.T     (weight-stationary, output transposed [feat, seq]),
     RoPE applied via a half-swap permutation matmul + two DVE multiplies.
  3. Banded attention: per (head, 128-query block) only the 3 key blocks
     covering the 256-wide window are computed; softmax without max-
     subtraction (scores bounded by construction), software-pipelined.
  4. out = att @ wo.T  (weight-stationary, transposed output; host
     untransposes).

Host/runtime architecture (the wall-clock bottleneck — the device program is
~300us; everything else is axon-tunnel transfer + dispatch):

  - All jax/device work lives in SPAWNED WORKER PROCESSES — 4 workers x 2
    cores each, because the tunnel's ~32MB/s cap is per client connection:
    four connections fetch the output slices at ~77MB/s aggregate. The axon
    terminal is flaky (episodic "mesh desynced / NRT_EXEC_UNIT_UNRECOVERABLE"
    that an in-process PJRT rebuild cannot clear, while a fresh process claim
    can), so the parent supervises each worker with timeouts and respawns it
    on any error or hang, then re-stages and re-runs just that worker.
  - The worker builds one jitted shard_map executable and keeps all staged
    inputs device-resident across calls; the previous call's output buffer
    is donated back as the next call's (fully overwritten) output
    allocation. outT is bf16, halving the only per-call fetch.
  - The parent content-hashes the inputs concurrently with the optimistic
    device run + fetch, and only re-stages when an input actually changed.
  - Results return through a shared-memory segment (one 33.5MB f32 copy).
"""

import atexit
import hashlib
import math
import os
import sys
import time
from concurrent.futures import ThreadPoolExecutor
from multiprocessing import shared_memory

import numpy as np
import ml_dtypes

# the spawned worker resolves this module by name; make sure its directory
# is importable in the child even if the parent imported it by file path
_MODDIR = os.path.dirname(os.path.abspath(__file__))
if _MODDIR not in sys.path:
    sys.path.insert(0, _MODDIR)

B, S, D = 2, 2048, 2048
H = 16
HD = 128
W = 256
NCORES = 8
SC = 512            # query positions per core
KV = SC + W         # 768 key/value positions per core
NDB = D // 128      # 16 contraction blocks
SCALE = 1.0 / math.sqrt(HD)

bf16 = ml_dtypes.bfloat16

OUT_BYTES = B * S * D * 4
_IN_NAMES = ("x", "fc", "fs", "wq", "wk", "wv", "wo")

# The axon tunnel's ~32MB/s cap is per client connection (BDP-limited);
# several worker processes fetch in parallel at ~48MB/s (2) / ~77MB/s (4)
# aggregate. 4 workers x 2 cores each is the sweet spot (8 concurrent
# claims overload the pool's claim path).
N_WORKERS = 4
CORES_PER = NCORES // N_WORKERS

_CACHE = {}

# ======================================================================
# device program (runs in the worker process)
# ======================================================================


def _build_program(repeat=1, parts=(1, 2, 3)):
    import concourse.bass as bass
    import concourse.mybir as mybir
    import concourse.tile as tile

    BF16 = mybir.dt.bfloat16
    FP32 = mybir.dt.float32
    Exp = mybir.ActivationFunctionType.Exp

    nc = bass.Bass()

    xT = nc.declare_dram_parameter("xT", [128, NDB, KV], BF16, isOutput=False)
    wqt = nc.declare_dram_parameter("wqt", [H, 128, NDB, 128], BF16, isOutput=False)
    wkt = nc.declare_dram_parameter("wkt", [H, 128, NDB, 128], BF16, isOutput=False)
    wvt = nc.declare_dram_parameter("wvt", [4, 128, NDB, 512], BF16, isOutput=False)
    wot = nc.declare_dram_parameter("wot", [16, 128, NDB, 128], BF16, isOutput=False)
    tabc = nc.declare_dram_parameter("tabc", [128, KV], BF16, isOutput=False)
    tabsn = nc.declare_dram_parameter("tabsn", [128, KV], BF16, isOutput=False)
    masks = nc.declare_dram_parameter("masks", [128, 12, 128], BF16, isOutput=False)
    p128 = nc.declare_dram_parameter("p128", [128, 128], BF16, isOutput=False)
    i128 = nc.declare_dram_parameter("i128", [128, 128], BF16, isOutput=False)
    outT = nc.declare_dram_parameter("outT", [D, SC], BF16, isOutput=True)

    with tile.TileContext(nc) as tc:
        with tc.tile_pool(name="const", bufs=1) as singles, \
             tc.tile_pool(name="wts", bufs=1) as wpool, \
             tc.tile_pool(name="rope", bufs=1) as rpool, \
             tc.tile_pool(name="att", bufs=1) as apool, \
             tc.tile_pool(name="outp", bufs=1) as opool, \
             tc.tile_pool(name="dscratch", bufs=1, space="DRAM") as dpool:

            # --- resident inputs / constants ---
            xT_sb = singles.tile([128, NDB, KV], BF16)
            nc.gpsimd.dma_start(out=xT_sb[:, 0, 0:128], in_=xT[:, 0, 0:128])
            nc.gpsimd.dma_start(out=xT_sb[:, 0, 128:KV], in_=xT[:, 0, 128:KV])
            for db in range(1, NDB):
                nc.gpsimd.dma_start(out=xT_sb[:, db, :], in_=xT[:, db, :])
            tabc_sb = singles.tile([128, KV], BF16)
            nc.gpsimd.dma_start(out=tabc_sb, in_=tabc[:, :])
            tabs_sb = singles.tile([128, KV], BF16)
            nc.gpsimd.dma_start(out=tabs_sb, in_=tabsn[:, :])
            masks_sb = singles.tile([128, 12, 128], BF16)
            nc.gpsimd.dma_start(out=masks_sb, in_=masks[:, :, :])
            p_sb = singles.tile([128, 128], BF16)
            nc.gpsimd.dma_start(out=p_sb, in_=p128[:, :])
            i_sb = singles.tile([128, 128], BF16)
            nc.gpsimd.dma_start(out=i_sb, in_=i128[:, :])
            ones_col = singles.tile([128, 1], BF16)
            nc.vector.memset(ones_col, 1.0)
            ones_row = singles.tile([1, 128], FP32)
            nc.vector.memset(ones_row, 1.0)

            # --- resident intermediates ---
            k_sb = singles.tile([128, H, KV], BF16)      # [hd, h, key pos]
            q_sb = singles.tile([128, H, SC], BF16)      # [hd, h, query pos]
            v_sb = singles.tile([128, KV // 128, D], BF16)  # [pos%128, pos//128, feat]
            att_sb = singles.tile([128, H, SC], BF16)    # [hd, h, query pos]

            # PE warmup: trivial matmuls on resident constants fill the
            # initial input-DMA wait and flip the HAM clock gate to 2.4GHz
            # before the first real matmul. Scratch psum, no readers.
            with tc.tile_pool(name="warm", bufs=1, space="PSUM") as warmp:
                wps = warmp.tile([1, 2], FP32, name="warm_ps")
                for _ in range(32):
                    nc.tensor.matmul(
                        wps[0:1, 0:1], lhsT=ones_col, rhs=ones_col,
                        start=True, stop=True,
                    )

            def _phases(rep):
                # ---------------- phase 1: projections ----------------
                if 2 in parts and 1 not in parts:
                    nc.vector.memset(k_sb, 0.01)
                    nc.vector.memset(q_sb, 0.01)
                    nc.vector.memset(v_sb, 0.01)
                if 3 in parts and 2 not in parts:
                    nc.vector.memset(att_sb, 0.01)
                if 1 in parts:
                    _p1(rep)
                if 2 in parts:
                    _p2(rep)
                if 3 in parts:
                    _p3(rep)

            def _p1(rep):
                with tc.tile_pool(name=f"pp1_{rep}", bufs=1, space="PSUM") as pp1:
                    # V projection: x-stationary, normal [seq, feat] output
                    for oc in range(4):
                        wv_t = wpool.tile([128, NDB, 512], BF16, tag="wv", bufs=2)
                        for db in range(NDB):
                            nc.sync.dma_start(out=wv_t[:, db, :], in_=wvt[oc, :, db, :])
                        for rb in range(KV // 128):
                            ps = pp1.tile([128, 512], FP32, tag="big", bufs=4)
                            for db in range(NDB):
                                nc.tensor.matmul(
                                    ps,
                                    lhsT=xT_sb[:, db, rb * 128:(rb + 1) * 128],
                                    rhs=wv_t[:, db, :],
                                    start=(db == 0),
                                    stop=(db == NDB - 1),
                                )
                            nc.scalar.copy(
                                out=v_sb[:, rb, oc * 512:(oc + 1) * 512], in_=ps
                            )

                    # K and Q projections: weight-stationary, transposed output
                    def proj_rope(w_dram, dst, dst_off, r0, rn):
                        # dst[:, h, dst_off:dst_off+rn] = RoPE(w.T @ x[:, r0:r0+rn])
                        for h in range(H):
                            w_t = wpool.tile([128, NDB, 128], BF16, tag="wqk", bufs=4)
                            nc.sync.dma_start(out=w_t, in_=w_dram[h])
                            for c0 in range(0, rn, 512):
                                cn = min(512, rn - c0)
                                a0 = r0 + c0          # column offset into xT / tabs
                                ps = pp1.tile([128, 512], FP32, tag="big", bufs=4)
                                for db in range(NDB):
                                    nc.tensor.matmul(
                                        ps[:, :cn],
                                        lhsT=w_t[:, db, :],
                                        rhs=xT_sb[:, db, a0:a0 + cn],
                                        start=(db == 0),
                                        stop=(db == NDB - 1),
                                    )
                                raw = rpool.tile([128, 512], BF16, tag="raw", bufs=4)
                                nc.scalar.copy(out=raw[:, :cn], in_=ps[:, :cn])
                                tc_ = rpool.tile([128, 512], BF16, tag="tc", bufs=4)
                                nc.vector.tensor_mul(
                                    tc_[:, :cn], raw[:, :cn], tabc_sb[:, a0:a0 + cn]
                                )
                                # swap(q) * S2 == swap(q * swap(S2)): multiply
                                # by the pre-swapped sine table, then swap the
                                # 64-partition halves with two SBUF DMAs.
                                us = rpool.tile([128, 512], BF16, tag="us", bufs=4)
                                nc.vector.tensor_mul(
                                    us[:, :cn], raw[:, :cn], tabs_sb[:, a0:a0 + cn]
                                )
                                sw = rpool.tile([128, 512], BF16, tag="sw", bufs=4)
                                nc.sync.dma_start(
                                    out=sw[0:64, :cn], in_=us[64:128, :cn]
                                )
                                nc.sync.dma_start(
                                    out=sw[64:128, :cn], in_=us[0:64, :cn]
                                )
                                o0 = dst_off + c0
                                nc.vector.tensor_add(
                                    dst[:, h, o0:o0 + cn], tc_[:, :cn], sw[:, :cn]
                                )

                    proj_rope(wkt, k_sb, 0, 0, KV)
                    proj_rope(wqt, q_sb, 0, W, SC)

            def _p2(rep):
                # ---------------- phase 2: banded attention ----------------
                # Software-pipelined: for pair i, the exp/mask (ACT/DVE) of
                # pair i runs while PE already issues QK of pair i+1; the
                # ones/PV matmuls of pair i follow. Normalization (recip +
                # DRAM-bounce broadcast + final muls) trails one head.
                with tc.tile_pool(name=f"pp2_{rep}", bufs=1, space="PSUM") as pp2:
                    pairs = [(h, t) for h in range(H) for t in range(4)]
                    state = {}   # live tiles per pair index
                    heads = {}   # h -> {"d": ps_d, "araws": [...]}
                    pending = []

                    def stage_a(i):
                        h, t = pairs[i]
                        ps_s = pp2.tile([128, 3, 128], FP32, tag="s", bufs=3)
                        # additive {0,-30000} mask seeds the psum via an
                        # identity matmul; the QK matmuls then accumulate into
                        # their 128-column regions and close the group.
                        nc.tensor.matmul(
                            ps_s,
                            lhsT=i_sb,
                            rhs=masks_sb[:, t * 3:t * 3 + 3, :],
                            start=True,
                            stop=False,
                            skip_group_check=True,
                        )
                        for blk in range(3):
                            kb = t + blk
                            nc.tensor.matmul(
                                ps_s[:, blk, :],
                                lhsT=k_sb[:, h, kb * 128:(kb + 1) * 128],
                                rhs=q_sb[:, h, t * 128:(t + 1) * 128],
                                start=False,
                                stop=True,
                                skip_group_check=True,
                            )
                        e = apool.tile([128, 3, 128], BF16, tag="e", bufs=5)
                        nc.scalar.activation(e, ps_s, Exp, scale=SCALE)
                        state[i] = e

                    def stage_b(i):
                        h, t = pairs[i]
                        em = state.pop(i)  # e tile (mask already applied)
                        if t == 0:
                            heads[h] = {
                                "d": pp2.tile([1, 512], FP32, tag="d", bufs=2, name="ps_d"),
                                "araws": [],
                            }
                        hs = heads[h]
                        for blk in range(3):
                            nc.tensor.matmul(
                                hs["d"][:, t * 128:(t + 1) * 128],
                                lhsT=ones_col,
                                rhs=em[:, blk, :],
                                start=(blk == 0),
                                stop=(blk == 2),
                            )
                        ps_pv = pp2.tile([128, 128], FP32, tag="pv", bufs=3)
                        for blk in range(3):
                            nc.tensor.matmul(
                                ps_pv,
                                lhsT=v_sb[:, t + blk, h * 128:(h + 1) * 128],
                                rhs=em[:, blk, :],
                                start=(blk == 0),
                                stop=(blk == 2),
                            )
                        araw = apool.tile([128, 128], BF16, tag="araw", bufs=12)
                        nc.vector.tensor_copy(araw, ps_pv)
                        hs["araws"].append(araw)
                        if t == 3:
                            close_head(h)

                    def close_head(h):
                        hs = heads.pop(h)
                        r_sb = apool.tile([1, 512], FP32, tag="rinv", bufs=3)
                        nc.vector.reciprocal(r_sb, hs["d"])
                        rd = dpool.tile([1, 512], FP32, tag="rd", bufs=3)
                        nc.sync.dma_start(out=rd, in_=r_sb)
                        rbc = apool.tile([128, 512], FP32, tag="rbc", bufs=3)
                        nc.sync.dma_start(
                            out=rbc, in_=rd[:, :].to_broadcast([128, 512])
                        )
                        pending.append((h, hs["araws"], rbc))
                        if len(pending) > 1:
                            flush_pending()

                    def flush_pending():
                        hh, araws_p, rbc_p = pending.pop(0)
                        for tt in range(4):
                            nc.vector.tensor_mul(
                                att_sb[:, hh, tt * 128:(tt + 1) * 128],
                                araws_p[tt],
                                rbc_p[:, tt * 128:(tt + 1) * 128],
                            )

                    for i in range(len(pairs) + 1):
                        if i < len(pairs):
                            stage_a(i)
                        if i >= 1:
                            stage_b(i - 1)
                    while pending:
                        flush_pending()

            def _p3(rep):
                # ---------------- phase 3: output projection ----------------
                with tc.tile_pool(name=f"pp3_{rep}", bufs=1, space="PSUM") as pp3:
                    for ob in range(16):
                        wo_t = wpool.tile([128, NDB, 128], BF16, tag="wqk", bufs=4)
                        nc.sync.dma_start(out=wo_t, in_=wot[ob])
                        ps_o = pp3.tile([128, 512], FP32, tag="wo", bufs=3)
                        for fb in range(H):
                            nc.tensor.matmul(
                                ps_o,
                                lhsT=wo_t[:, fb, :],
                                rhs=att_sb[:, fb, :],
                                start=(fb == 0),
                                stop=(fb == H - 1),
                            )
                        o_stage = opool.tile([128, 512], BF16, tag="ostg", bufs=3)
                        nc.scalar.copy(out=o_stage, in_=ps_o)
                        eng = nc.sync if ob % 2 == 0 else nc.gpsimd
                        eng.dma_start(
                            out=outT[ob * 128:(ob + 1) * 128, :], in_=o_stage
                        )


            for _rep in range(repeat):
                _phases(_rep)

    return nc


def _split_multi_waits(nc, mybir, max_waits=1):
    """This walrus build encodes at most one sync-wait command per
    instruction; Tile attaches one wait per producing proc. Move extra waits
    onto same-engine NoOps inserted immediately before the instruction."""
    n_split = 0
    for f in nc.m.functions:
        for blk in f.blocks:
            ins_list = blk.instructions
            i = 0
            while i < len(ins_list):
                inst = ins_list[i]
                si = getattr(inst, "sync_info", None)
                waits = list(si.on_wait) if si is not None and si.on_wait else []
                if len(waits) > max_waits:
                    si.on_wait = waits[:max_waits]
                    rest = waits[max_waits:]
                    for k in range(0, len(rest), max_waits):
                        nop = mybir.InstNoOp(
                            name=f"{inst.name}_sw{k}",
                            engine=inst.engine,
                            sync_info=mybir.SyncInfo(
                                on_wait=rest[k : k + max_waits], on_update=[]
                            ),
                            bass_nofuse=True,
                        )
                        ins_list.insert(i, nop)
                        i += 1
                    n_split += 1
                i += 1
    return n_split


# ======================================================================
# host-side input prep (worker process)
# ======================================================================


def _prep_shared(wq, wk, wv, wo):
    """Host-side weight prep: head-feature permutation + tile-major layouts."""
    # permutation: within each head, feature 2f -> f (re), 2f+1 -> 64+f (im)
    perm = np.empty(D, dtype=np.int64)
    for h in range(H):
        base = h * HD
        perm[base:base + 64] = base + 2 * np.arange(64)
        perm[base + 64:base + 128] = base + 2 * np.arange(64) + 1

    def tiles_128(wt):  # wt: [d, o] -> [o_blk, p, d_blk, 128]
        return np.ascontiguousarray(
            wt.reshape(NDB, 128, 16, 128).transpose(2, 1, 0, 3)
        )

    wq_t = tiles_128(wq[perm].T.astype(bf16))
    wk_t = tiles_128(wk[perm].T.astype(bf16))
    wo_t = tiles_128(wo.T.astype(bf16))
    wv_t = np.ascontiguousarray(
        wv.T.astype(bf16).reshape(NDB, 128, 4, 512).transpose(2, 1, 0, 3)
    )
    return wq_t, wk_t, wv_t, wo_t


def _prep_consts():
    p = np.zeros((128, 128), dtype=bf16)
    p[np.arange(64) + 64, np.arange(64)] = 1.0
    p[np.arange(64), np.arange(64) + 64] = 1.0
    eye = np.eye(128, dtype=bf16)
    return p, eye


def _prep_x_tiles(x):
    """x [B,S,D] f32 -> per-core tiled bf16 [NCORES*128, NDB, KV]."""
    xb = x.astype(bf16)
    xpad = np.concatenate([np.zeros((B, W, D), dtype=bf16), xb], axis=1)
    cores = np.stack(
        [xpad[b, j * SC:j * SC + KV] for b in range(B) for j in range(4)]
    )  # [8, KV, D]
    xt = np.ascontiguousarray(cores.transpose(0, 2, 1))  # [8, D, KV]
    xt = xt.reshape(NCORES, NDB, 128, KV).transpose(0, 2, 1, 3)
    return np.ascontiguousarray(xt).reshape(NCORES * 128, NDB, KV)


def _prep_tabs(freqs_cos, freqs_sin, cores):
    """RoPE tables per core: [len(cores)*128, KV] each (cos, pre-swapped sine)."""
    tabc_parts, tabs_parts = [], []
    for c in cores:
        _, j = divmod(c, 4)
        s0 = j * SC
        g = np.clip(np.arange(s0 - W, s0 + SC), 0, S - 1)
        cos_g = freqs_cos[g].T.astype(bf16)          # [64, KV]
        sin_g = freqs_sin[g].T
        tabc_parts.append(np.concatenate([cos_g, cos_g], axis=0))
        tabs_parts.append(
            np.concatenate([sin_g, -sin_g], axis=0).astype(bf16)
        )
    return (np.ascontiguousarray(np.concatenate(tabc_parts, axis=0)),
            np.ascontiguousarray(np.concatenate(tabs_parts, axis=0)))


def _prep_masks(cores):
    """Additive {0,-30000} banded masks per core: [len(cores)*128, 12, 128]."""
    parts = []
    for c in cores:
        _, j = divmod(c, 4)
        s0 = j * SC
        kj = np.arange(128)[:, None, None]
        tb = np.arange(12)[None, :, None]
        qi = np.arange(128)[None, None, :]
        t, blk = tb // 3, tb % 3
        gq = s0 + 128 * t + qi
        gk = s0 - W + 128 * (t + blk) + kj
        valid = (gk >= 0) & (gk <= gq) & (gk > gq - W)
        parts.append(np.where(valid, 0.0, -30000.0).astype(bf16))
    return np.ascontiguousarray(np.concatenate(parts, axis=0))


# ======================================================================
# worker process: owns jax + the axon devices
# ======================================================================


class _Runner:
    """Builds the SPMD executable once and keeps all staged inputs device-
    resident across calls. Mirrors concourse.bass2jax.run_bass_via_pjrt but
    with a persistent jit, device-resident staging, and output-buffer
    recycling (outT is fully overwritten by the kernel, so no zero-init is
    needed)."""

    def __init__(self, nc, pool, base, ncores):
        import jax
        import concourse.mybir as mybir
        from concourse.bass2jax import (
            _bass_exec_p, install_neuronx_cc_hook, partition_id_tensor,
        )
        from jax.experimental.shard_map import shard_map
        from jax.sharding import Mesh, PartitionSpec, NamedSharding

        install_neuronx_cc_hook()
        self.jax = jax
        self.nc = nc
        self.pool = pool
        self.base = base          # first global core index of this worker
        self.ncores = ncores      # cores driven by this worker
        assert nc.dbg_addr is None
        partition_name = (nc.partition_id_tensor.name
                          if nc.partition_id_tensor else None)

        in_names, out_names, out_avals = [], [], []
        for alloc in nc.m.functions[0].allocations:
            if not isinstance(alloc, mybir.MemoryLocationSet):
                continue
            name = alloc.memorylocations[0].name
            if alloc.kind == "ExternalInput":
                if name != partition_name:
                    in_names.append(name)
            elif alloc.kind == "ExternalOutput":
                out_names.append(name)
                out_avals.append(jax.core.ShapedArray(
                    tuple(alloc.tensor_shape), mybir.dt.np(alloc.dtype)))
        self.in_names = list(in_names)
        self.out_names = list(out_names)
        self.out_avals = out_avals
        n_params = len(in_names)
        n_outs = len(out_names)
        all_names = in_names + out_names
        if partition_name is not None:
            all_names = all_names + [partition_name]

        def _body(*args):
            operands = list(args)
            if partition_name is not None:
                operands.append(partition_id_tensor())
            outs = _bass_exec_p.bind(
                *operands,
                out_avals=tuple(out_avals),
                in_names=tuple(all_names),
                out_names=tuple(out_names),
                lowering_input_output_aliases=(),
                sim_require_finite=True,
                sim_require_nnan=True,
                nc=nc,
            )
            return tuple(outs)

        devices = jax.devices()[self.base:self.base + self.ncores]
        assert len(devices) == self.ncores
        self.mesh = Mesh(np.asarray(devices), ("core",))
        self.devices = devices
        self.sharding = NamedSharding(self.mesh, PartitionSpec("core"))
        in_specs = (PartitionSpec("core"),) * (n_params + n_outs)
        out_specs = (PartitionSpec("core"),) * n_outs
        self.sharded = jax.jit(
            shard_map(_body, mesh=self.mesh, in_specs=in_specs,
                      out_specs=out_specs, check_rep=False),
            donate_argnums=tuple(range(n_params, n_params + n_outs)),
            keep_unused=True,
        )
        self.staged = {}        # input name -> committed device array
        self.staged_key = {}    # cache-group name -> content-hash key
        self.recycle = None     # previous call's outT device array

    def put(self, name, global_np):
        """Upload [ncores*s0, ...] host array as a sharded device array."""
        jax = self.jax
        s0 = global_np.shape[0] // self.ncores
        parts = [global_np[c * s0:(c + 1) * s0] for c in range(self.ncores)]
        futs = [self.pool.submit(jax.device_put, p, d)
                for p, d in zip(parts, self.devices)]
        arrs = [f.result() for f in futs]
        self.staged[name] = jax.make_array_from_single_device_arrays(
            global_np.shape, self.sharding, arrs)

    def out_buffer(self):
        jax = self.jax
        if self.recycle is not None:
            buf, self.recycle = self.recycle, None
            return buf
        aval = self.out_avals[0]
        z = np.zeros((self.ncores * aval.shape[0],) + tuple(aval.shape[1:]),
                     aval.dtype)
        return jax.device_put(z, self.sharding)

    def dispatch(self):
        """Launch the device program asynchronously; returns the out array."""
        (out,) = self.sharded(*[self.staged[n] for n in self.in_names],
                              self.out_buffer())
        return out

    def fetch_finalize(self, out, res):
        """Fetch the 8 outT shards in parallel, untransposing and upcasting
        each into res [B,4,SC,D] f32 as its bytes arrive; keep the device
        buffer for donation."""
        shards = out.addressable_shards
        for s in shards:             # kick off all 8 device->host streams
            try:
                s.data.copy_to_host_async()
            except Exception:
                pass

        def work(s):
            c = self.base + (s.index[0].start or 0) // D
            b, j = divmod(c, 4)
            res[b, j] = np.asarray(s.data).T

        list(self.pool.map(work, shards))
        self.recycle = out


def _worker_stage(r, groups):
    """Apply staging payloads {group: {"key":..., "arrays": {...}}}."""
    cores = list(range(r.base, r.base + r.ncores))
    nw = r.ncores
    if "const" in groups:
        r.put("masks", _prep_masks(cores))
        p, eye = _prep_consts()
        r.put("p128", np.ascontiguousarray(np.broadcast_to(
            p, (nw,) + p.shape)).reshape(nw * 128, 128))
        r.put("i128", np.ascontiguousarray(np.broadcast_to(
            eye, (nw,) + eye.shape)).reshape(nw * 128, 128))
        r.staged_key["const"] = groups["const"]["key"]
    if "w" in groups:
        a = groups["w"]["arrays"]
        wq_t, wk_t, wv_t, wo_t = _prep_shared(
            a["wq"], a["wk"], a["wv"], a["wo"])
        for name, t in [("wqt", wq_t), ("wkt", wk_t),
                        ("wvt", wv_t), ("wot", wo_t)]:
            g = np.ascontiguousarray(
                np.broadcast_to(t, (nw,) + t.shape)
            ).reshape((nw * t.shape[0],) + t.shape[1:])
            r.put(name, g)
        r.staged_key["w"] = groups["w"]["key"]
    if "f" in groups:
        a = groups["f"]["arrays"]
        tabc_g, tabs_g = _prep_tabs(a["fc"], a["fs"], cores)
        r.put("tabc", tabc_g)
        r.put("tabsn", tabs_g)
        r.staged_key["f"] = groups["f"]["key"]
    if "x" in groups:
        full = _prep_x_tiles(groups["x"]["arrays"]["x"])
        r.put("xT", np.ascontiguousarray(
            full[r.base * 128:(r.base + nw) * 128]))
        r.staged_key["x"] = groups["x"]["key"]


def _worker_entry():
    """Child-process entry: connect back to the parent and serve requests."""
    from multiprocessing.connection import Client

    addr = os.environ["SWA_SOCK"]
    key = bytes.fromhex(os.environ["SWA_AUTH"])
    conn = Client(addr, authkey=key)
    _worker_main(conn, os.environ["SWA_SHM"])


def _worker_main(conn, shm_name):
    """Request loop of the device-worker process."""
    try:       # track=False: don't let this child's resource tracker unlink
        shm = shared_memory.SharedMemory(name=shm_name, track=False)
    except TypeError:  # python < 3.13
        shm = shared_memory.SharedMemory(name=shm_name)
    res = np.ndarray((B, 4, SC, D), dtype=np.float32, buffer=shm.buf)
    base = int(os.environ.get("SWA_BASE", "0"))
    ncores = int(os.environ.get("SWA_NCORES", str(NCORES)))
    pool = ThreadPoolExecutor(8)
    state = {"runner": None}

    def runner():
        if state["runner"] is None:
            import concourse.mybir as mybir

            nc = _build_program()
            _split_multi_waits(nc, mybir)
            state["runner"] = _Runner(nc, pool, base, ncores)
        return state["runner"]

    while True:
        try:
            msg = conn.recv()
        except (EOFError, OSError):
            break
        try:
            cmd = msg["cmd"]
            if cmd == "stage":
                r = runner()
                _worker_stage(r, msg["groups"])
                conn.send({"ok": True, "keys": dict(r.staged_key)})
            elif cmd == "run":
                r = runner()
                out = r.dispatch()
                r.fetch_finalize(out, res)
                conn.send({"ok": True, "keys": dict(r.staged_key)})
            elif cmd == "ping":
                conn.send({"ok": True})
            elif cmd == "exit":
                conn.send({"ok": True})
                break
            else:
                conn.send({"ok": False, "err": f"unknown cmd {cmd}"})
        except BaseException as e:  # noqa: BLE001 — parent decides what's next
            try:
                conn.send({"ok": False, "err": repr(e)})
            except Exception:
                break


# ======================================================================
# parent process: supervision, hashing, result copy
# ======================================================================


class _WorkerDied(RuntimeError):
    pass


class _ConsistencyError(_WorkerDied):
    pass


def _hash(arr):
    a = np.ascontiguousarray(arr)
    return hashlib.blake2b(memoryview(a).cast("B"), digest_size=16).digest()


class _Worker:
    __slots__ = ("idx", "proc", "conn", "keys", "ran_once")

    def __init__(self, idx):
        self.idx = idx
        self.proc = None
        self.conn = None
        self.keys = None          # staged group keys (None = fresh worker)
        self.ran_once = False

    @property
    def alive(self):
        return (self.proc is not None and self.proc.poll() is None
                and self.conn is not None)


class _Supervisor:
    def __init__(self):
        self.workers = [_Worker(i) for i in range(N_WORKERS)]
        name = f"swa_out_{os.getpid()}"

        def _mk(**kw):
            try:   # track=False: manage unlink ourselves, skip the tracker
                return shared_memory.SharedMemory(
                    create=True, size=OUT_BYTES, name=name, track=False, **kw)
            except TypeError:  # python < 3.13
                return shared_memory.SharedMemory(
                    create=True, size=OUT_BYTES, name=name, **kw)

        try:
            self.shm = _mk()
        except FileExistsError:
            try:
                shared_memory.SharedMemory(name=name).unlink()
            except Exception:
                pass
            self.shm = _mk()
        atexit.register(self._cleanup)
        self.res_view = np.ndarray((B, 4, SC, D), dtype=np.float32,
                                   buffer=self.shm.buf)
        self.hash_pool = ThreadPoolExecutor(4)
        self._spawned = 0
        # cross-call consistency: (group-keys tuple, output sample) of the
        # last successful call; detects silently corrupted device staging
        self.last_sig = None
        self.check_fuse = 2

    def _cleanup(self):
        for w in self.workers:
            try:
                self.kill(w)
            except Exception:
                pass
        try:
            self.shm.close()
            self.shm.unlink()
        except Exception:
            pass

    def spawn(self, w):
        self.kill(w)
        import secrets
        import subprocess
        from multiprocessing.connection import Listener

        self._spawned += 1
        addr = f"/tmp/swa_sock_{os.getpid()}_{self._spawned}"
        try:
            os.unlink(addr)
        except FileNotFoundError:
            pass
        authkey = secrets.token_bytes(16)
        listener = Listener(address=addr, family="AF_UNIX", authkey=authkey)
        env = dict(os.environ)
        env["SWA_SOCK"] = addr
        env["SWA_SHM"] = self.shm.name
        env["SWA_AUTH"] = authkey.hex()
        env["SWA_BASE"] = str(w.idx * CORES_PER)
        env["SWA_NCORES"] = str(CORES_PER)
        w.proc = subprocess.Popen(
            [sys.executable, "-c", "import kernel; kernel._worker_entry()"],
            cwd=_MODDIR, env=env)
        try:
            fut = self.hash_pool.submit(listener.accept)
            w.conn = fut.result(timeout=300)
        except Exception as e:
            listener.close()
            self.kill(w)
            raise _WorkerDied(f"worker{w.idx} failed to connect: {e!r}") from e
        listener.close()
        try:
            os.unlink(addr)
        except FileNotFoundError:
            pass
        w.keys = None
        w.ran_once = False

    def ensure_all(self):
        for w in self.workers:
            if not w.alive:
                self.spawn(w)

    def kill(self, w):
        try:
            if w.proc is not None and w.proc.poll() is None:
                w.proc.kill()
                w.proc.wait(timeout=10)
        except Exception:
            pass
        try:
            if w.conn is not None:
                w.conn.close()
        except Exception:
            pass
        w.proc = None
        w.conn = None
        w.keys = None
        w.ran_once = False

    def send(self, w, msg):
        try:
            w.conn.send(msg)
        except Exception as e:
            self.kill(w)
            raise _WorkerDied(f"worker{w.idx} send: {e!r}") from e

    def wait(self, w, timeout, what="run"):
        try:
            if not w.conn.poll(timeout):
                raise _WorkerDied(f"worker{w.idx} timeout on {what}")
            rep = w.conn.recv()
        except _WorkerDied:
            self.kill(w)
            raise
        except Exception as e:
            self.kill(w)
            raise _WorkerDied(f"worker{w.idx} {what}: {e!r}") from e
        if not rep.get("ok"):
            self.kill(w)
            raise _WorkerDied(f"worker{w.idx} {what}: {rep.get('err')}")
        return rep

    def result(self):
        return np.array(self.res_view, copy=True).reshape(B, S, D)


def _group_keys(hs):
    return {
        "const": b"static",
        "w": (hs["wq"], hs["wk"], hs["wv"], hs["wo"]),
        "f": (hs["fc"], hs["fs"]),
        "x": hs["x"],
    }


def _stage_payload(gkeys, arrs, groups):
    x, fc, fs, wq, wk, wv, wo = arrs
    payload = {}
    for g in groups:
        if g == "const":
            payload[g] = {"key": gkeys[g], "arrays": {}}
        elif g == "w":
            payload[g] = {"key": gkeys[g],
                          "arrays": {"wq": wq, "wk": wk, "wv": wv, "wo": wo}}
        elif g == "f":
            payload[g] = {"key": gkeys[g], "arrays": {"fc": fc, "fs": fs}}
        elif g == "x":
            payload[g] = {"key": gkeys[g], "arrays": {"x": x}}
    return payload


_RUN_TIMEOUT = 180.0          # steady-state run (includes 16.8MB fetch)
_FIRST_RUN_TIMEOUT = 2400.0   # first run: neuronxcc compile + executable load
_STAGE_TIMEOUT = 1200.0       # staging: jax/program build + ~300MB upload


def _sample(res):
    flat = res.reshape(-1)
    return flat[::flat.size // 16384].copy()


def _checked(sup, gkeys, res):
    """Same inputs must reproduce the same output (the device program is
    deterministic); a mismatch means device staging was silently corrupted —
    recover through the respawn path. Fused off after two firings so a
    surprise (e.g. nondeterminism) can never loop."""
    sig = (gkeys["const"], gkeys["w"], gkeys["f"], gkeys["x"])
    smp = _sample(res)
    if (sup.check_fuse > 0 and sup.last_sig is not None
            and sup.last_sig[0] == sig
            and not np.array_equal(sup.last_sig[1], smp)):
        sup.check_fuse -= 1
        sup.last_sig = None
        raise _ConsistencyError("output mismatch for identical inputs")
    sup.last_sig = (sig, smp)
    return res


def _stage_workers(sup, ws, gkeys, arrs):
    """Stage each worker's missing/stale groups (sends first, then waits,
    so the workers prep+upload concurrently)."""
    staged = []
    for w in ws:
        groups = (list(gkeys) if w.keys is None else
                  [g for g in gkeys if w.keys.get(g) != gkeys[g]])
        if not groups:
            continue
        sup.send(w, {"cmd": "stage",
                     "groups": _stage_payload(gkeys, arrs, groups)})
        staged.append(w)
    for w in staged:
        rep = sup.wait(w, _STAGE_TIMEOUT, "stage")
        w.keys = rep["keys"]


def _run_workers(sup, ws):
    """Run the device program on the given workers; returns failed ones."""
    sent, failed = [], []
    for w in ws:
        try:
            sup.send(w, {"cmd": "run"})
            sent.append(w)
        except _WorkerDied:
            failed.append(w)
    for w in sent:
        try:
            sup.wait(w, _RUN_TIMEOUT if w.ran_once else _FIRST_RUN_TIMEOUT)
            w.ran_once = True
        except _WorkerDied:
            failed.append(w)
    return failed


def _recover(sup, ws, gkeys, arrs):
    """Respawn+restage+rerun the given workers; raises if any still fail."""
    for w in ws:
        if not w.alive:
            sup.spawn(w)
    _stage_workers(sup, ws, gkeys, arrs)
    failed = _run_workers(sup, ws)
    if failed:
        raise _WorkerDied(f"workers {[w.idx for w in failed]} failed")


def _orchestrate(sup, arrs):
    sup.ensure_all()
    if any(w.keys is None for w in sup.workers):
        # at least one fresh worker: hash now, stage, run everyone
        hs = dict(zip(_IN_NAMES, sup.hash_pool.map(_hash, arrs)))
        gkeys = _group_keys(hs)
        _stage_workers(sup, sup.workers, gkeys, arrs)
        failed = _run_workers(sup, sup.workers)
        if failed:
            _recover(sup, failed, gkeys, arrs)
        return _checked(sup, gkeys, sup.result())

    # all workers warm: optimistic run with current staging; hash overlapped
    hfuts = [sup.hash_pool.submit(_hash, a) for a in arrs]
    failed = _run_workers(sup, sup.workers)
    hs = dict(zip(_IN_NAMES, (f.result() for f in hfuts)))
    gkeys = _group_keys(hs)
    rerun = [w for w in sup.workers
             if w in failed or w.keys is None
             or any(w.keys.get(g) != gkeys[g] for g in gkeys)]
    if rerun:
        _recover(sup, rerun, gkeys, arrs)
    return _checked(sup, gkeys, sup.result())


def kernel(x, freqs_cos, freqs_sin, wq, wk, wv, wo):
    arrs = [np.ascontiguousarray(np.asarray(a, dtype=np.float32))
            for a in (x, freqs_cos, freqs_sin, wq, wk, wv, wo)]

    if "sup" not in _CACHE:
        _CACHE["sup"] = _Supervisor()
    sup = _CACHE["sup"]

    last = None
    for attempt in range(4):
        try:
            return _orchestrate(sup, arrs)
        except _ConsistencyError as e:  # corrupted staging: full respawn
            last = e
            for w in sup.workers:
                sup.kill(w)
            time.sleep(2.0 * (attempt + 1))
        except _WorkerDied as e:   # flaky axon terminal: fresh claims + retry
            last = e
            for w in sup.workers:
                if not w.alive or w.keys is None:
                    sup.kill(w)
            time.sleep(2.0 * (attempt + 1))
    raise last


# revision 39
# speedup vs baseline: 1.1624x; 1.1624x over previous
"""Sliding-window attention (B=2, S=2048, D=2048, H=16, HD=128, W=256) on 8
Trainium2 NeuronCores.

Sharding: data-parallel on batch (2) x sequence-parallel (4 chunks of 512
queries). Each core recomputes the K/V projections for its 256-position halo,
so there are no collectives; the host gathers the 8 output slices.

Per-core pipeline (all matmuls bf16 with f32 PSUM accumulation):
  1. V = x @ wv.T      (x-stationary, output in [seq, feat] layout)
  2. K,Q = x @ w.T     (weight-stationary, output transposed [feat, seq]),
     RoPE applied via a half-swap permutation matmul + two DVE multiplies.
  3. Banded attention: per (head, 128-query block) only the 3 key blocks
     covering the 256-wide window are computed; softmax without max-
     subtraction (scores bounded by construction), software-pipelined.
  4. out = att @ wo.T  (weight-stationary, transposed output; host
     untransposes).

Host/runtime architecture (the wall-clock bottleneck — the device program is
~300us; everything else is axon-tunnel transfer + dispatch):

  - All jax/device work lives in a SPAWNED WORKER PROCESS. The axon terminal
    is flaky (episodic "mesh desynced / NRT_EXEC_UNIT_UNRECOVERABLE" that an
    in-process PJRT rebuild cannot clear, while a fresh process claim can),
    so the parent supervises the worker with timeouts and respawns it on any
    error or hang, then re-stages and re-runs.
  - The worker builds one jitted shard_map executable and keeps all staged
    inputs device-resident across calls; the previous call's output buffer
    is donated back as the next call's (fully overwritten) output
    allocation. outT is bf16, halving the only per-call fetch.
  - The parent content-hashes the inputs concurrently with the optimistic
    device run + fetch, and only re-stages when an input actually changed.
  - Results return through a shared-memory segment (one 33.5MB f32 copy).
"""

import atexit
import hashlib
import math
import os
import sys
import time
from concurrent.futures import ThreadPoolExecutor
from multiprocessing import shared_memory

import numpy as np
import ml_dtypes

# the spawned worker resolves this module by name; make sure its directory
# is importable in the child even if the parent imported it by file path
_MODDIR = os.path.dirname(os.path.abspath(__file__))
if _MODDIR not in sys.path:
    sys.path.insert(0, _MODDIR)

B, S, D = 2, 2048, 2048
H = 16
HD = 128
W = 256
NCORES = 8
SC = 512            # query positions per core
KV = SC + W         # 768 key/value positions per core
NDB = D // 128      # 16 contraction blocks
SCALE = 1.0 / math.sqrt(HD)

bf16 = ml_dtypes.bfloat16

OUT_BYTES = B * S * D * 4
_IN_NAMES = ("x", "fc", "fs", "wq", "wk", "wv", "wo")

# One worker process driving all 8 cores. Multi-worker sharding (tested with
# 4 workers x 2 cores, each fetching its output slice over its own client
# connection) does NOT help: the terminal's device->host staging path is a
# shared ~30MB/s bottleneck that already pipelines with the tunnel leg, so
# aggregate fetch bandwidth is flat in the number of connections for
# fresh-off-device results. A single worker has the best min latency, one
# compile, and the smallest flake surface.
N_WORKERS = 1
CORES_PER = NCORES // N_WORKERS

_CACHE = {}

# ======================================================================
# device program (runs in the worker process)
# ======================================================================


def _build_program(repeat=1, parts=(1, 2, 3)):
    import concourse.bass as bass
    import concourse.mybir as mybir
    import concourse.tile as tile

    BF16 = mybir.dt.bfloat16
    FP32 = mybir.dt.float32
    Exp = mybir.ActivationFunctionType.Exp

    nc = bass.Bass()

    xT = nc.declare_dram_parameter("xT", [128, NDB, KV], BF16, isOutput=False)
    wqt = nc.declare_dram_parameter("wqt", [H, 128, NDB, 128], BF16, isOutput=False)
    wkt = nc.declare_dram_parameter("wkt", [H, 128, NDB, 128], BF16, isOutput=False)
    wvt = nc.declare_dram_parameter("wvt", [4, 128, NDB, 512], BF16, isOutput=False)
    wot = nc.declare_dram_parameter("wot", [16, 128, NDB, 128], BF16, isOutput=False)
    tabc = nc.declare_dram_parameter("tabc", [128, KV], BF16, isOutput=False)
    tabsn = nc.declare_dram_parameter("tabsn", [128, KV], BF16, isOutput=False)
    masks = nc.declare_dram_parameter("masks", [128, 12, 128], BF16, isOutput=False)
    p128 = nc.declare_dram_parameter("p128", [128, 128], BF16, isOutput=False)
    i128 = nc.declare_dram_parameter("i128", [128, 128], BF16, isOutput=False)
    outT = nc.declare_dram_parameter("outT", [D, SC], BF16, isOutput=True)

    with tile.TileContext(nc) as tc:
        with tc.tile_pool(name="const", bufs=1) as singles, \
             tc.tile_pool(name="wts", bufs=1) as wpool, \
             tc.tile_pool(name="rope", bufs=1) as rpool, \
             tc.tile_pool(name="att", bufs=1) as apool, \
             tc.tile_pool(name="outp", bufs=1) as opool, \
             tc.tile_pool(name="dscratch", bufs=1, space="DRAM") as dpool:

            # --- resident inputs / constants ---
            xT_sb = singles.tile([128, NDB, KV], BF16)
            nc.gpsimd.dma_start(out=xT_sb[:, 0, 0:128], in_=xT[:, 0, 0:128])
            nc.gpsimd.dma_start(out=xT_sb[:, 0, 128:KV], in_=xT[:, 0, 128:KV])
            for db in range(1, NDB):
                nc.gpsimd.dma_start(out=xT_sb[:, db, :], in_=xT[:, db, :])
            tabc_sb = singles.tile([128, KV], BF16)
            nc.gpsimd.dma_start(out=tabc_sb, in_=tabc[:, :])
            tabs_sb = singles.tile([128, KV], BF16)
            nc.gpsimd.dma_start(out=tabs_sb, in_=tabsn[:, :])
            masks_sb = singles.tile([128, 12, 128], BF16)
            nc.gpsimd.dma_start(out=masks_sb, in_=masks[:, :, :])
            p_sb = singles.tile([128, 128], BF16)
            nc.gpsimd.dma_start(out=p_sb, in_=p128[:, :])
            i_sb = singles.tile([128, 128], BF16)
            nc.gpsimd.dma_start(out=i_sb, in_=i128[:, :])
            ones_col = singles.tile([128, 1], BF16)
            nc.vector.memset(ones_col, 1.0)
            ones_row = singles.tile([1, 128], FP32)
            nc.vector.memset(ones_row, 1.0)

            # --- resident intermediates ---
            k_sb = singles.tile([128, H, KV], BF16)      # [hd, h, key pos]
            q_sb = singles.tile([128, H, SC], BF16)      # [hd, h, query pos]
            v_sb = singles.tile([128, KV // 128, D], BF16)  # [pos%128, pos//128, feat]
            att_sb = singles.tile([128, H, SC], BF16)    # [hd, h, query pos]

            # PE warmup: trivial matmuls on resident constants fill the
            # initial input-DMA wait and flip the HAM clock gate to 2.4GHz
            # before the first real matmul. Scratch psum, no readers.
            with tc.tile_pool(name="warm", bufs=1, space="PSUM") as warmp:
                wps = warmp.tile([1, 2], FP32, name="warm_ps")
                for _ in range(32):
                    nc.tensor.matmul(
                        wps[0:1, 0:1], lhsT=ones_col, rhs=ones_col,
                        start=True, stop=True,
                    )

            def _phases(rep):
                # ---------------- phase 1: projections ----------------
                if 2 in parts and 1 not in parts:
                    nc.vector.memset(k_sb, 0.01)
                    nc.vector.memset(q_sb, 0.01)
                    nc.vector.memset(v_sb, 0.01)
                if 3 in parts and 2 not in parts:
                    nc.vector.memset(att_sb, 0.01)
                if 1 in parts:
                    _p1(rep)
                if 2 in parts:
                    _p2(rep)
                if 3 in parts:
                    _p3(rep)

            def _p1(rep):
                with tc.tile_pool(name=f"pp1_{rep}", bufs=1, space="PSUM") as pp1:
                    # V projection: x-stationary, normal [seq, feat] output
                    for oc in range(4):
                        wv_t = wpool.tile([128, NDB, 512], BF16, tag="wv", bufs=2)
                        for db in range(NDB):
                            nc.sync.dma_start(out=wv_t[:, db, :], in_=wvt[oc, :, db, :])
                        for rb in range(KV // 128):
                            ps = pp1.tile([128, 512], FP32, tag="big", bufs=4)
                            for db in range(NDB):
                                nc.tensor.matmul(
                                    ps,
                                    lhsT=xT_sb[:, db, rb * 128:(rb + 1) * 128],
                                    rhs=wv_t[:, db, :],
                                    start=(db == 0),
                                    stop=(db == NDB - 1),
                                )
                            nc.scalar.copy(
                                out=v_sb[:, rb, oc * 512:(oc + 1) * 512], in_=ps
                            )

                    # K and Q projections: weight-stationary, transposed output
                    def proj_rope(w_dram, dst, dst_off, r0, rn):
                        # dst[:, h, dst_off:dst_off+rn] = RoPE(w.T @ x[:, r0:r0+rn])
                        for h in range(H):
                            w_t = wpool.tile([128, NDB, 128], BF16, tag="wqk", bufs=4)
                            nc.sync.dma_start(out=w_t, in_=w_dram[h])
                            for c0 in range(0, rn, 512):
                                cn = min(512, rn - c0)
                                a0 = r0 + c0          # column offset into xT / tabs
                                ps = pp1.tile([128, 512], FP32, tag="big", bufs=4)
                                for db in range(NDB):
                                    nc.tensor.matmul(
                                        ps[:, :cn],
                                        lhsT=w_t[:, db, :],
                                        rhs=xT_sb[:, db, a0:a0 + cn],
                                        start=(db == 0),
                                        stop=(db == NDB - 1),
                                    )
                                raw = rpool.tile([128, 512], BF16, tag="raw", bufs=4)
                                nc.scalar.copy(out=raw[:, :cn], in_=ps[:, :cn])
                                tc_ = rpool.tile([128, 512], BF16, tag="tc", bufs=4)
                                nc.vector.tensor_mul(
                                    tc_[:, :cn], raw[:, :cn], tabc_sb[:, a0:a0 + cn]
                                )
                                # swap(q) * S2 == swap(q * swap(S2)): multiply
                                # by the pre-swapped sine table, then swap the
                                # 64-partition halves with two SBUF DMAs.
                                us = rpool.tile([128, 512], BF16, tag="us", bufs=4)
                                nc.vector.tensor_mul(
                                    us[:, :cn], raw[:, :cn], tabs_sb[:, a0:a0 + cn]
                                )
                                sw = rpool.tile([128, 512], BF16, tag="sw", bufs=4)
                                nc.sync.dma_start(
                                    out=sw[0:64, :cn], in_=us[64:128, :cn]
                                )
                                nc.sync.dma_start(
                                    out=sw[64:128, :cn], in_=us[0:64, :cn]
                                )
                                o0 = dst_off + c0
                                nc.vector.tensor_add(
                                    dst[:, h, o0:o0 + cn], tc_[:, :cn], sw[:, :cn]
                                )

                    proj_rope(wkt, k_sb, 0, 0, KV)
                    proj_rope(wqt, q_sb, 0, W, SC)

            def _p2(rep):
                # ---------------- phase 2: banded attention ----------------
                # Software-pipelined: for pair i, the exp/mask (ACT/DVE) of
                # pair i runs while PE already issues QK of pair i+1; the
                # ones/PV matmuls of pair i follow. Normalization (recip +
                # DRAM-bounce broadcast + final muls) trails one head.
                with tc.tile_pool(name=f"pp2_{rep}", bufs=1, space="PSUM") as pp2:
                    pairs = [(h, t) for h in range(H) for t in range(4)]
                    state = {}   # live tiles per pair index
                    heads = {}   # h -> {"d": ps_d, "araws": [...]}
                    pending = []

                    def stage_a(i):
                        h, t = pairs[i]
                        ps_s = pp2.tile([128, 3, 128], FP32, tag="s", bufs=3)
                        # additive {0,-30000} mask seeds the psum via an
                        # identity matmul; the QK matmuls then accumulate into
                        # their 128-column regions and close the group.
                        nc.tensor.matmul(
                            ps_s,
                            lhsT=i_sb,
                            rhs=masks_sb[:, t * 3:t * 3 + 3, :],
                            start=True,
                            stop=False,
                            skip_group_check=True,
                        )
                        for blk in range(3):
                            kb = t + blk
                            nc.tensor.matmul(
                                ps_s[:, blk, :],
                                lhsT=k_sb[:, h, kb * 128:(kb + 1) * 128],
                                rhs=q_sb[:, h, t * 128:(t + 1) * 128],
                                start=False,
                                stop=True,
                                skip_group_check=True,
                            )
                        e = apool.tile([128, 3, 128], BF16, tag="e", bufs=5)
                        nc.scalar.activation(e, ps_s, Exp, scale=SCALE)
                        state[i] = e

                    def stage_b(i):
                        h, t = pairs[i]
                        em = state.pop(i)  # e tile (mask already applied)
                        if t == 0:
                            heads[h] = {
                                "d": pp2.tile([1, 512], FP32, tag="d", bufs=2, name="ps_d"),
                                "araws": [],
                            }
                        hs = heads[h]
                        for blk in range(3):
                            nc.tensor.matmul(
                                hs["d"][:, t * 128:(t + 1) * 128],
                                lhsT=ones_col,
                                rhs=em[:, blk, :],
                                start=(blk == 0),
                                stop=(blk == 2),
                            )
                        ps_pv = pp2.tile([128, 128], FP32, tag="pv", bufs=3)
                        for blk in range(3):
                            nc.tensor.matmul(
                                ps_pv,
                                lhsT=v_sb[:, t + blk, h * 128:(h + 1) * 128],
                                rhs=em[:, blk, :],
                                start=(blk == 0),
                                stop=(blk == 2),
                            )
                        araw = apool.tile([128, 128], BF16, tag="araw", bufs=12)
                        nc.vector.tensor_copy(araw, ps_pv)
                        hs["araws"].append(araw)
                        if t == 3:
                            close_head(h)

                    def close_head(h):
                        hs = heads.pop(h)
                        r_sb = apool.tile([1, 512], FP32, tag="rinv", bufs=3)
                        nc.vector.reciprocal(r_sb, hs["d"])
                        rd = dpool.tile([1, 512], FP32, tag="rd", bufs=3)
                        nc.sync.dma_start(out=rd, in_=r_sb)
                        rbc = apool.tile([128, 512], FP32, tag="rbc", bufs=3)
                        nc.sync.dma_start(
                            out=rbc, in_=rd[:, :].to_broadcast([128, 512])
                        )
                        pending.append((h, hs["araws"], rbc))
                        if len(pending) > 1:
                            flush_pending()

                    def flush_pending():
                        hh, araws_p, rbc_p = pending.pop(0)
                        for tt in range(4):
                            nc.vector.tensor_mul(
                                att_sb[:, hh, tt * 128:(tt + 1) * 128],
                                araws_p[tt],
                                rbc_p[:, tt * 128:(tt + 1) * 128],
                            )

                    for i in range(len(pairs) + 1):
                        if i < len(pairs):
                            stage_a(i)
                        if i >= 1:
                            stage_b(i - 1)
                    while pending:
                        flush_pending()

            def _p3(rep):
                # ---------------- phase 3: output projection ----------------
                with tc.tile_pool(name=f"pp3_{rep}", bufs=1, space="PSUM") as pp3:
                    for ob in range(16):
                        wo_t = wpool.tile([128, NDB, 128], BF16, tag="wqk", bufs=4)
                        nc.sync.dma_start(out=wo_t, in_=wot[ob])
                        ps_o = pp3.tile([128, 512], FP32, tag="wo", bufs=3)
                        for fb in range(H):
                            nc.tensor.matmul(
                                ps_o,
                                lhsT=wo_t[:, fb, :],
                                rhs=att_sb[:, fb, :],
                                start=(fb == 0),
                                stop=(fb == H - 1),
                            )
                        o_stage = opool.tile([128, 512], BF16, tag="ostg", bufs=3)
                        nc.scalar.copy(out=o_stage, in_=ps_o)
                        eng = nc.sync if ob % 2 == 0 else nc.gpsimd
                        eng.dma_start(
                            out=outT[ob * 128:(ob + 1) * 128, :], in_=o_stage
                        )


            for _rep in range(repeat):
                _phases(_rep)

    return nc


def _split_multi_waits(nc, mybir, max_waits=1):
    """This walrus build encodes at most one sync-wait command per
    instruction; Tile attaches one wait per producing proc. Move extra waits
    onto same-engine NoOps inserted immediately before the instruction."""
    n_split = 0
    for f in nc.m.functions:
        for blk in f.blocks:
            ins_list = blk.instructions
            i = 0
            while i < len(ins_list):
                inst = ins_list[i]
                si = getattr(inst, "sync_info", None)
                waits = list(si.on_wait) if si is not None and si.on_wait else []
                if len(waits) > max_waits:
                    si.on_wait = waits[:max_waits]
                    rest = waits[max_waits:]
                    for k in range(0, len(rest), max_waits):
                        nop = mybir.InstNoOp(
                            name=f"{inst.name}_sw{k}",
                            engine=inst.engine,
                            sync_info=mybir.SyncInfo(
                                on_wait=rest[k : k + max_waits], on_update=[]
                            ),
                            bass_nofuse=True,
                        )
                        ins_list.insert(i, nop)
                        i += 1
                    n_split += 1
                i += 1
    return n_split


# ======================================================================
# host-side input prep (worker process)
# ======================================================================


def _prep_shared(wq, wk, wv, wo):
    """Host-side weight prep: head-feature permutation + tile-major layouts."""
    # permutation: within each head, feature 2f -> f (re), 2f+1 -> 64+f (im)
    perm = np.empty(D, dtype=np.int64)
    for h in range(H):
        base = h * HD
        perm[base:base + 64] = base + 2 * np.arange(64)
        perm[base + 64:base + 128] = base + 2 * np.arange(64) + 1

    def tiles_128(wt):  # wt: [d, o] -> [o_blk, p, d_blk, 128]
        return np.ascontiguousarray(
            wt.reshape(NDB, 128, 16, 128).transpose(2, 1, 0, 3)
        )

    wq_t = tiles_128(wq[perm].T.astype(bf16))
    wk_t = tiles_128(wk[perm].T.astype(bf16))
    wo_t = tiles_128(wo.T.astype(bf16))
    wv_t = np.ascontiguousarray(
        wv.T.astype(bf16).reshape(NDB, 128, 4, 512).transpose(2, 1, 0, 3)
    )
    return wq_t, wk_t, wv_t, wo_t


def _prep_consts():
    p = np.zeros((128, 128), dtype=bf16)
    p[np.arange(64) + 64, np.arange(64)] = 1.0
    p[np.arange(64), np.arange(64) + 64] = 1.0
    eye = np.eye(128, dtype=bf16)
    return p, eye


def _prep_x_tiles(x):
    """x [B,S,D] f32 -> per-core tiled bf16 [NCORES*128, NDB, KV]."""
    xb = x.astype(bf16)
    xpad = np.concatenate([np.zeros((B, W, D), dtype=bf16), xb], axis=1)
    cores = np.stack(
        [xpad[b, j * SC:j * SC + KV] for b in range(B) for j in range(4)]
    )  # [8, KV, D]
    xt = np.ascontiguousarray(cores.transpose(0, 2, 1))  # [8, D, KV]
    xt = xt.reshape(NCORES, NDB, 128, KV).transpose(0, 2, 1, 3)
    return np.ascontiguousarray(xt).reshape(NCORES * 128, NDB, KV)


def _prep_tabs(freqs_cos, freqs_sin, cores):
    """RoPE tables per core: [len(cores)*128, KV] each (cos, pre-swapped sine)."""
    tabc_parts, tabs_parts = [], []
    for c in cores:
        _, j = divmod(c, 4)
        s0 = j * SC
        g = np.clip(np.arange(s0 - W, s0 + SC), 0, S - 1)
        cos_g = freqs_cos[g].T.astype(bf16)          # [64, KV]
        sin_g = freqs_sin[g].T
        tabc_parts.append(np.concatenate([cos_g, cos_g], axis=0))
        tabs_parts.append(
            np.concatenate([sin_g, -sin_g], axis=0).astype(bf16)
        )
    return (np.ascontiguousarray(np.concatenate(tabc_parts, axis=0)),
            np.ascontiguousarray(np.concatenate(tabs_parts, axis=0)))


def _prep_masks(cores):
    """Additive {0,-30000} banded masks per core: [len(cores)*128, 12, 128]."""
    parts = []
    for c in cores:
        _, j = divmod(c, 4)
        s0 = j * SC
        kj = np.arange(128)[:, None, None]
        tb = np.arange(12)[None, :, None]
        qi = np.arange(128)[None, None, :]
        t, blk = tb // 3, tb % 3
        gq = s0 + 128 * t + qi
        gk = s0 - W + 128 * (t + blk) + kj
        valid = (gk >= 0) & (gk <= gq) & (gk > gq - W)
        parts.append(np.where(valid, 0.0, -30000.0).astype(bf16))
    return np.ascontiguousarray(np.concatenate(parts, axis=0))


# ======================================================================
# worker process: owns jax + the axon devices
# ======================================================================


class _Runner:
    """Builds the SPMD executable once and keeps all staged inputs device-
    resident across calls. Mirrors concourse.bass2jax.run_bass_via_pjrt but
    with a persistent jit, device-resident staging, and output-buffer
    recycling (outT is fully overwritten by the kernel, so no zero-init is
    needed)."""

    def __init__(self, nc, pool, base, ncores):
        import jax
        import concourse.mybir as mybir
        from concourse.bass2jax import (
            _bass_exec_p, install_neuronx_cc_hook, partition_id_tensor,
        )
        from jax.experimental.shard_map import shard_map
        from jax.sharding import Mesh, PartitionSpec, NamedSharding

        install_neuronx_cc_hook()
        self.jax = jax
        self.nc = nc
        self.pool = pool
        self.base = base          # first global core index of this worker
        self.ncores = ncores      # cores driven by this worker
        assert nc.dbg_addr is None
        partition_name = (nc.partition_id_tensor.name
                          if nc.partition_id_tensor else None)

        in_names, out_names, out_avals = [], [], []
        for alloc in nc.m.functions[0].allocations:
            if not isinstance(alloc, mybir.MemoryLocationSet):
                continue
            name = alloc.memorylocations[0].name
            if alloc.kind == "ExternalInput":
                if name != partition_name:
                    in_names.append(name)
            elif alloc.kind == "ExternalOutput":
                out_names.append(name)
                out_avals.append(jax.core.ShapedArray(
                    tuple(alloc.tensor_shape), mybir.dt.np(alloc.dtype)))
        self.in_names = list(in_names)
        self.out_names = list(out_names)
        self.out_avals = out_avals
        n_params = len(in_names)
        n_outs = len(out_names)
        all_names = in_names + out_names
        if partition_name is not None:
            all_names = all_names + [partition_name]

        def _body(*args):
            operands = list(args)
            if partition_name is not None:
                operands.append(partition_id_tensor())
            outs = _bass_exec_p.bind(
                *operands,
                out_avals=tuple(out_avals),
                in_names=tuple(all_names),
                out_names=tuple(out_names),
                lowering_input_output_aliases=(),
                sim_require_finite=True,
                sim_require_nnan=True,
                nc=nc,
            )
            return tuple(outs)

        devices = jax.devices()[self.base:self.base + self.ncores]
        assert len(devices) == self.ncores
        self.mesh = Mesh(np.asarray(devices), ("core",))
        self.devices = devices
        self.sharding = NamedSharding(self.mesh, PartitionSpec("core"))
        in_specs = (PartitionSpec("core"),) * (n_params + n_outs)
        out_specs = (PartitionSpec("core"),) * n_outs
        self.sharded = jax.jit(
            shard_map(_body, mesh=self.mesh, in_specs=in_specs,
                      out_specs=out_specs, check_rep=False),
            donate_argnums=tuple(range(n_params, n_params + n_outs)),
            keep_unused=True,
        )
        self.staged = {}        # input name -> committed device array
        self.staged_key = {}    # cache-group name -> content-hash key
        self.recycle = None     # previous call's outT device array

    def put(self, name, global_np):
        """Upload [ncores*s0, ...] host array as a sharded device array."""
        jax = self.jax
        s0 = global_np.shape[0] // self.ncores
        parts = [global_np[c * s0:(c + 1) * s0] for c in range(self.ncores)]
        futs = [self.pool.submit(jax.device_put, p, d)
                for p, d in zip(parts, self.devices)]
        arrs = [f.result() for f in futs]
        self.staged[name] = jax.make_array_from_single_device_arrays(
            global_np.shape, self.sharding, arrs)

    def out_buffer(self):
        jax = self.jax
        if self.recycle is not None:
            buf, self.recycle = self.recycle, None
            return buf
        aval = self.out_avals[0]
        z = np.zeros((self.ncores * aval.shape[0],) + tuple(aval.shape[1:]),
                     aval.dtype)
        return jax.device_put(z, self.sharding)

    def dispatch(self):
        """Launch the device program asynchronously; returns the out array."""
        (out,) = self.sharded(*[self.staged[n] for n in self.in_names],
                              self.out_buffer())
        return out

    def fetch_finalize(self, out, res):
        """Fetch the 8 outT shards in parallel, untransposing and upcasting
        each into res [B,4,SC,D] f32 as its bytes arrive; keep the device
        buffer for donation."""
        shards = out.addressable_shards
        for s in shards:             # kick off all 8 device->host streams
            try:
                s.data.copy_to_host_async()
            except Exception:
                pass

        def work(s):
            c = self.base + (s.index[0].start or 0) // D
            b, j = divmod(c, 4)
            res[b, j] = np.asarray(s.data).T

        list(self.pool.map(work, shards))
        self.recycle = out


def _worker_stage(r, groups):
    """Apply staging payloads {group: {"key":..., "arrays": {...}}}."""
    cores = list(range(r.base, r.base + r.ncores))
    nw = r.ncores
    if "const" in groups:
        r.put("masks", _prep_masks(cores))
        p, eye = _prep_consts()
        r.put("p128", np.ascontiguousarray(np.broadcast_to(
            p, (nw,) + p.shape)).reshape(nw * 128, 128))
        r.put("i128", np.ascontiguousarray(np.broadcast_to(
            eye, (nw,) + eye.shape)).reshape(nw * 128, 128))
        r.staged_key["const"] = groups["const"]["key"]
    if "w" in groups:
        a = groups["w"]["arrays"]
        wq_t, wk_t, wv_t, wo_t = _prep_shared(
            a["wq"], a["wk"], a["wv"], a["wo"])
        for name, t in [("wqt", wq_t), ("wkt", wk_t),
                        ("wvt", wv_t), ("wot", wo_t)]:
            g = np.ascontiguousarray(
                np.broadcast_to(t, (nw,) + t.shape)
            ).reshape((nw * t.shape[0],) + t.shape[1:])
            r.put(name, g)
        r.staged_key["w"] = groups["w"]["key"]
    if "f" in groups:
        a = groups["f"]["arrays"]
        tabc_g, tabs_g = _prep_tabs(a["fc"], a["fs"], cores)
        r.put("tabc", tabc_g)
        r.put("tabsn", tabs_g)
        r.staged_key["f"] = groups["f"]["key"]
    if "x" in groups:
        full = _prep_x_tiles(groups["x"]["arrays"]["x"])
        r.put("xT", np.ascontiguousarray(
            full[r.base * 128:(r.base + nw) * 128]))
        r.staged_key["x"] = groups["x"]["key"]


def _worker_entry():
    """Child-process entry: connect back to the parent and serve requests."""
    from multiprocessing.connection import Client

    addr = os.environ["SWA_SOCK"]
    key = bytes.fromhex(os.environ["SWA_AUTH"])
    conn = Client(addr, authkey=key)
    _worker_main(conn, os.environ["SWA_SHM"])


def _worker_main(conn, shm_name):
    """Request loop of the device-worker process."""
    try:       # track=False: don't let this child's resource tracker unlink
        shm = shared_memory.SharedMemory(name=shm_name, track=False)
    except TypeError:  # python < 3.13
        shm = shared_memory.SharedMemory(name=shm_name)
    res = np.ndarray((B, 4, SC, D), dtype=np.float32, buffer=shm.buf)
    base = int(os.environ.get("SWA_BASE", "0"))
    ncores = int(os.environ.get("SWA_NCORES", str(NCORES)))
    pool = ThreadPoolExecutor(8)
    state = {"runner": None}

    def runner():
        if state["runner"] is None:
            import concourse.mybir as mybir

            nc = _build_program()
            _split_multi_waits(nc, mybir)
            state["runner"] = _Runner(nc, pool, base, ncores)
        return state["runner"]

    while True:
        try:
            msg = conn.recv()
        except (EOFError, OSError):
            break
        try:
            cmd = msg["cmd"]
            if cmd == "stage":
                r = runner()
                _worker_stage(r, msg["groups"])
                conn.send({"ok": True, "keys": dict(r.staged_key)})
            elif cmd == "run":
                t0 = time.time()
                r = runner()
                out = r.dispatch()
                t1 = time.time()
                r.fetch_finalize(out, res)
                t2 = time.time()
                conn.send({"ok": True, "keys": dict(r.staged_key),
                           "dt": (t1 - t0, t2 - t1)})
            elif cmd == "ping":
                conn.send({"ok": True})
            elif cmd == "exit":
                conn.send({"ok": True})
                break
            else:
                conn.send({"ok": False, "err": f"unknown cmd {cmd}"})
        except BaseException as e:  # noqa: BLE001 — parent decides what's next
            try:
                conn.send({"ok": False, "err": repr(e)})
            except Exception:
                break


# ======================================================================
# parent process: supervision, hashing, result copy
# ======================================================================


class _WorkerDied(RuntimeError):
    pass


class _ConsistencyError(_WorkerDied):
    pass


def _hash(arr):
    a = np.ascontiguousarray(arr)
    return hashlib.blake2b(memoryview(a).cast("B"), digest_size=16).digest()


class _Worker:
    __slots__ = ("idx", "proc", "conn", "keys", "ran_once")

    def __init__(self, idx):
        self.idx = idx
        self.proc = None
        self.conn = None
        self.keys = None          # staged group keys (None = fresh worker)
        self.ran_once = False

    @property
    def alive(self):
        return (self.proc is not None and self.proc.poll() is None
                and self.conn is not None)


class _Supervisor:
    def __init__(self):
        self.workers = [_Worker(i) for i in range(N_WORKERS)]
        name = f"swa_out_{os.getpid()}"

        def _mk(**kw):
            try:   # track=False: manage unlink ourselves, skip the tracker
                return shared_memory.SharedMemory(
                    create=True, size=OUT_BYTES, name=name, track=False, **kw)
            except TypeError:  # python < 3.13
                return shared_memory.SharedMemory(
                    create=True, size=OUT_BYTES, name=name, **kw)

        try:
            self.shm = _mk()
        except FileExistsError:
            try:
                shared_memory.SharedMemory(name=name).unlink()
            except Exception:
                pass
            self.shm = _mk()
        atexit.register(self._cleanup)
        self.res_view = np.ndarray((B, 4, SC, D), dtype=np.float32,
                                   buffer=self.shm.buf)
        self.hash_pool = ThreadPoolExecutor(4)
        self._spawned = 0
        # cross-call consistency: (group-keys tuple, output sample) of the
        # last successful call; detects silently corrupted device staging
        self.last_sig = None
        self.check_fuse = 2

    def _cleanup(self):
        for w in self.workers:
            try:
                self.kill(w)
            except Exception:
                pass
        try:
            self.shm.close()
            self.shm.unlink()
        except Exception:
            pass

    def spawn(self, w):
        self.kill(w)
        import secrets
        import subprocess
        from multiprocessing.connection import Listener

        self._spawned += 1
        addr = f"/tmp/swa_sock_{os.getpid()}_{self._spawned}"
        try:
            os.unlink(addr)
        except FileNotFoundError:
            pass
        authkey = secrets.token_bytes(16)
        listener = Listener(address=addr, family="AF_UNIX", authkey=authkey)
        env = dict(os.environ)
        env["SWA_SOCK"] = addr
        env["SWA_SHM"] = self.shm.name
        env["SWA_AUTH"] = authkey.hex()
        env["SWA_BASE"] = str(w.idx * CORES_PER)
        env["SWA_NCORES"] = str(CORES_PER)
        w.proc = subprocess.Popen(
            [sys.executable, "-c", "import kernel; kernel._worker_entry()"],
            cwd=_MODDIR, env=env)
        try:
            fut = self.hash_pool.submit(listener.accept)
            w.conn = fut.result(timeout=300)
        except Exception as e:
            listener.close()
            self.kill(w)
            raise _WorkerDied(f"worker{w.idx} failed to connect: {e!r}") from e
        listener.close()
        try:
            os.unlink(addr)
        except FileNotFoundError:
            pass
        w.keys = None
        w.ran_once = False

    def ensure_all(self):
        for w in self.workers:
            if not w.alive:
                self.spawn(w)

    def kill(self, w):
        try:
            if w.proc is not None and w.proc.poll() is None:
                w.proc.kill()
                w.proc.wait(timeout=10)
        except Exception:
            pass
        try:
            if w.conn is not None:
                w.conn.close()
        except Exception:
            pass
        w.proc = None
        w.conn = None
        w.keys = None
        w.ran_once = False

    def send(self, w, msg):
        try:
            w.conn.send(msg)
        except Exception as e:
            self.kill(w)
            raise _WorkerDied(f"worker{w.idx} send: {e!r}") from e

    def wait(self, w, timeout, what="run"):
        try:
            if not w.conn.poll(timeout):
                raise _WorkerDied(f"worker{w.idx} timeout on {what}")
            rep = w.conn.recv()
        except _WorkerDied:
            self.kill(w)
            raise
        except Exception as e:
            self.kill(w)
            raise _WorkerDied(f"worker{w.idx} {what}: {e!r}") from e
        if not rep.get("ok"):
            self.kill(w)
            raise _WorkerDied(f"worker{w.idx} {what}: {rep.get('err')}")
        return rep

    def result(self):
        return np.array(self.res_view, copy=True).reshape(B, S, D)


def _group_keys(hs):
    return {
        "const": b"static",
        "w": (hs["wq"], hs["wk"], hs["wv"], hs["wo"]),
        "f": (hs["fc"], hs["fs"]),
        "x": hs["x"],
    }


def _stage_payload(gkeys, arrs, groups):
    x, fc, fs, wq, wk, wv, wo = arrs
    payload = {}
    for g in groups:
        if g == "const":
            payload[g] = {"key": gkeys[g], "arrays": {}}
        elif g == "w":
            payload[g] = {"key": gkeys[g],
                          "arrays": {"wq": wq, "wk": wk, "wv": wv, "wo": wo}}
        elif g == "f":
            payload[g] = {"key": gkeys[g], "arrays": {"fc": fc, "fs": fs}}
        elif g == "x":
            payload[g] = {"key": gkeys[g], "arrays": {"x": x}}
    return payload


_RUN_TIMEOUT = 180.0          # steady-state run (includes 16.8MB fetch)
_FIRST_RUN_TIMEOUT = 2400.0   # first run: neuronxcc compile + executable load
_STAGE_TIMEOUT = 1200.0       # staging: jax/program build + ~300MB upload


def _sample(res):
    flat = res.reshape(-1)
    return flat[::flat.size // 16384].copy()


def _checked(sup, gkeys, res):
    """Same inputs must reproduce the same output (the device program is
    deterministic); a mismatch means device staging was silently corrupted —
    recover through the respawn path. Fused off after two firings so a
    surprise (e.g. nondeterminism) can never loop."""
    sig = (gkeys["const"], gkeys["w"], gkeys["f"], gkeys["x"])
    smp = _sample(res)
    if (sup.check_fuse > 0 and sup.last_sig is not None
            and sup.last_sig[0] == sig
            and not np.array_equal(sup.last_sig[1], smp)):
        sup.check_fuse -= 1
        sup.last_sig = None
        raise _ConsistencyError("output mismatch for identical inputs")
    sup.last_sig = (sig, smp)
    return res


def _stage_workers(sup, ws, gkeys, arrs):
    """Stage each worker's missing/stale groups (sends first, then waits,
    so the workers prep+upload concurrently)."""
    staged = []
    for w in ws:
        groups = (list(gkeys) if w.keys is None else
                  [g for g in gkeys if w.keys.get(g) != gkeys[g]])
        if not groups:
            continue
        sup.send(w, {"cmd": "stage",
                     "groups": _stage_payload(gkeys, arrs, groups)})
        staged.append(w)
    for w in staged:
        rep = sup.wait(w, _STAGE_TIMEOUT, "stage")
        w.keys = rep["keys"]


def _run_workers(sup, ws):
    """Run the device program on the given workers; returns failed ones."""
    sent, failed = [], []
    for w in ws:
        try:
            sup.send(w, {"cmd": "run"})
            sent.append(w)
        except _WorkerDied:
            failed.append(w)
    for w in sent:
        try:
            sup.wait(w, _RUN_TIMEOUT if w.ran_once else _FIRST_RUN_TIMEOUT)
            w.ran_once = True
        except _WorkerDied:
            failed.append(w)
    return failed


def _recover(sup, ws, gkeys, arrs):
    """Respawn+restage+rerun the given workers; raises if any still fail."""
    for w in ws:
        if not w.alive:
            sup.spawn(w)
    _stage_workers(sup, ws, gkeys, arrs)
    failed = _run_workers(sup, ws)
    if failed:
        raise _WorkerDied(f"workers {[w.idx for w in failed]} failed")


def _orchestrate(sup, arrs):
    sup.ensure_all()
    if any(w.keys is None for w in sup.workers):
        # at least one fresh worker: hash now, stage, run everyone
        hs = dict(zip(_IN_NAMES, sup.hash_pool.map(_hash, arrs)))
        gkeys = _group_keys(hs)
        _stage_workers(sup, sup.workers, gkeys, arrs)
        failed = _run_workers(sup, sup.workers)
        if failed:
            _recover(sup, failed, gkeys, arrs)
        return _checked(sup, gkeys, sup.result())

    # all workers warm: optimistic run with current staging; hash overlapped
    hfuts = [sup.hash_pool.submit(_hash, a) for a in arrs]
    failed = _run_workers(sup, sup.workers)
    hs = dict(zip(_IN_NAMES, (f.result() for f in hfuts)))
    gkeys = _group_keys(hs)
    rerun = [w for w in sup.workers
             if w in failed or w.keys is None
             or any(w.keys.get(g) != gkeys[g] for g in gkeys)]
    if rerun:
        _recover(sup, rerun, gkeys, arrs)
    return _checked(sup, gkeys, sup.result())


def kernel(x, freqs_cos, freqs_sin, wq, wk, wv, wo):
    arrs = [np.ascontiguousarray(np.asarray(a, dtype=np.float32))
            for a in (x, freqs_cos, freqs_sin, wq, wk, wv, wo)]

    if "sup" not in _CACHE:
        _CACHE["sup"] = _Supervisor()
    sup = _CACHE["sup"]

    last = None
    for attempt in range(4):
        try:
            return _orchestrate(sup, arrs)
        except _ConsistencyError as e:  # corrupted staging: full respawn
            last = e
            for w in sup.workers:
                sup.kill(w)
            time.sleep(2.0 * (attempt + 1))
        except _WorkerDied as e:   # flaky axon terminal: fresh claims + retry
            last = e
            for w in sup.workers:
                if not w.alive or w.keys is None:
                    sup.kill(w)
            time.sleep(2.0 * (attempt + 1))
    raise last
